# revision 73
# baseline (speedup 1.0000x reference)
"""Causal dot-product attention (s=2048, b=4, h=16, d=128) on 8 TRN2 NeuronCores.

Sharding: batch*heads (64 pairs) split across 8 cores -> 8 (b,h) pairs per core.
Core c handles b = c // 2, heads h in [(c%2)*8, (c%2)*8 + 8).

Dual-engine exp (the baseline was ACT(exp)-throughput-bound at ~146us busy;
this design splits softmax work across ACT, DVE, Pool and PE):
  - ACT blocks (i5 0..2): exp via ACTIVATE (table exp).
  - DVE block (i5=3): exp via the Schraudolph bit-trick: Q is pre-scaled on
    the host by 1024*log2(e)/sqrt(d) so scores z = 1024*log2(e)*s_scaled;
    one DVE tensor_scalar computes int16(round(z + 17408)), bitcast fp16 =
    exp(s_scaled)*4 with a ~1.8% rms periodic ripple.  The 4x cancels in the
    softmax division.  Entire sq-blocks (softmax ROWS) stay on one engine so
    the ripple's constant bias cancels row-wise; only the longest rows
    (i5=3) go to DVE (3.8e-3 rel err measured).
  - A- and D-chunks interleave in program order so both engines stream
    concurrently (the PSUM score ring serializes chunk order, not engines).

Chunking: 512-col score chunks (1 PSUM bank each), ring of 4 + two live ctx
sets (A-block + D-block, 2 banks each) = 8 banks.  Per i5 the column stream
is [diag tiles trimmed to 1024][full j-tiles][the 256-col diag leftover].
PV accumulation flags follow the PSUM group semantics: an accumulation group
commits at stop=True and a start=True discards any open group in the bank,
so start fires only on each bank's first write.

PV is emitted PV_DEFER chunks behind QK/exp, and BEFORE each QK in program
order, so PE always has ready PV work queued while a QK waits for its ring
slot (no head-of-line stall).  Diagonal masks run on the otherwise-idle Pool
engine (affine_select, fp16 SBUF).  Epilogue per block: batched reciprocals
(2 rowsums per ctx tile, strided AP) + ONE broadcast tensor_mul per ctx tile
normalizing both subtiles into a VW-strided staging tile; out-DMAs alternate
between the gpsimd and sync queues so neither sequencer backs up.

Cold start: ~15 dummy PE matmuls release the HAM clock gate (PE boots at
half clock until ~3.4us of sustained activity) during the DMA prologue;
head 0's input is 3 large DMAs; V's DRAM layout matches the SBUF tile
byte-for-byte so each head's V is one clean 2D transfer (the old strided
layout cost ~4us of sync-sequencer descriptor generation per head).

Host-side layout prep: Q scaled by 1024*log2(e)/sqrt(d) and transposed to
[head, d, s] interleaved with K as four [Q_blk | K_blk] 1024-col slabs, fp16.
V cast to fp16 as [head, partition, j, e] with the softmax-denominator ones
column baked in.
"""

import sys

if "/opt/trn_rl_repo" not in sys.path:
    sys.path.insert(0, "/opt/trn_rl_repo")

import numpy as np

import concourse.bacc as bacc
import concourse.bass as bass
import concourse.mybir as mybir
import concourse.tile as tile
from concourse.bass_utils import run_bass_kernel_spmd

S, B, H, D = 2048, 4, 16, 128
N_CORES = 8
HPC = (B * H) // N_CORES  # heads per core = 8

PRE = float(1024.0 * np.log2(np.e) / np.sqrt(128.0))  # folded into Q host-side
EXPSCALE = float(np.log(2.0) / 1024.0)  # ACT exp scale on raw z
# fp16 exponent bias (15*1024) + 2*1024: E = exp(s)*4.  Chosen so the max
# observed score (z=13175, 8.9 sigma) stays below the fp16 inf bit pattern
# (bits < 31744) and the int16 write never saturates; the 4x factor cancels
# in the softmax division.
DVE_BIAS = 17408.0

SQ_BLK = 512
N_I = S // SQ_BLK  # 4 sq blocks per head
N_SK = S // 128  # 16 sk tiles per head
VW = 129  # V tile width incl. ones column
CW = 512  # score chunk width (cols); fp32 -> 2KB -> 1 PSUM bank
RING = 4  # score ring depth (chunks in flight; 4 banks)
PV_DEFER = 8  # PV emitted this many chunks behind QK/exp

DVE_I5 = {3}  # sq blocks whose exp runs on DVE (longest rows: best accuracy)


def is_dve(hh, i5):
    return i5 in DVE_I5
PE_WARMUP = True  # dummy matmuls at t=0 to release the HAM clock gate
HEAD0_FEW_DMAS = True  # head 0 input in 3 large DMAs
MASKS_ON_POOL = True  # diag masks via gpsimd affine_select (else DVE tri mul)


def q_col(i5, c):
    # qk host/SBUF layout: four 1024-col blocks [Q_i5(512) | K_{4*i5..4*i5+3}]
    return i5 * 1024 + c


def k_col(j):
    return (j // 4) * 1024 + 512 + (j % 4) * 128


def pack_chunks(i5):
    """Chunks of (j, dst, mv0, w): QK matmul for sk-tile j writes score cols
    [dst, dst+w) of the chunk from moving Q cols [mv0, mv0+w).  Stream order:
    1024 cols of trimmed diag tiles, then full j-tiles, then the leftover
    256-col diag piece; entries split at chunk (1024) and PSUM-bank (512)
    boundaries."""
    stream = [(4 * i5 + 0, 0, 512), (4 * i5 + 1, 128, 384), (4 * i5 + 3, 384, 128)]
    stream += [(j, 0, 512) for j in range(4 * i5)]
    stream += [(4 * i5 + 2, 256, 256)]
    chunks, cur, used = [], [], 0
    for j, mv0, w in stream:
        while w:
            if used == CW:
                chunks.append(cur)
                cur, used = [], 0
            take = min(w, 512 - used % 512)
            cur.append((j, used, mv0, take))
            used += take
            mv0 += take
            w -= take
    if cur:
        chunks.append(cur)
    return chunks


_CHUNKS = {i5: pack_chunks(i5) for i5 in range(N_I)}


def pieces_of(i5):
    """Ordered PV pieces (ci, col, tt, j, diag) + first/last piece index per
    subtile tt (for matmul start/stop flags)."""
    ps = []
    for ci, ents in enumerate(_CHUNKS[i5]):
        for j, dst, mv0, w in ents:
            for k in range(w // 128):
                tt = mv0 // 128 + k
                ps.append((ci, dst + k * 128, tt, j, tt == j - 4 * i5))
    # PSUM semantics: an accumulation group commits at stop=True; start=True
    # discards any open (un-stopped) group in the bank.  So: start only on
    # each BANK's first write, stop on each subtile's last write.
    first, last = {}, {}
    for idx, (ci, col, tt, j, dg) in enumerate(ps):
        first.setdefault(tt // 2, idx)
        last[tt] = idx
    return ps, first, last


_PIECES = {i5: pieces_of(i5) for i5 in range(N_I)}


def chunk_width(i5, ci):
    return sum(w for _, _, _, w in _CHUNKS[i5][ci])


def build_global_chunks():
    """Global chunk order: fair-merge the A-chunk stream (ACT blocks, all
    heads) and D-chunk stream (DVE blocks, all heads) by fractional column
    progress, so ring alternation never degenerates at head boundaries
    (only ONE stream tail in the whole kernel).  D is delayed ~2048 cols at
    the start (head 0's late slabs) tapering to a small negative shift at
    the end so the final chunks are the last head's small A block (rolling
    drain)."""
    A, Dl = [], []
    for hh in range(HPC):
        order = [0, 1, 2, 3] if hh + 1 < HPC else [3, 2, 1, 0]
        for i5 in order:
            if not is_dve(hh, i5):
                for ci in range(len(_CHUNKS[i5])):
                    A.append((hh, i5, ci))
        for i5 in sorted(i for i in range(N_I) if is_dve(hh, i)):
            for ci in range(len(_CHUNKS[i5])):
                Dl.append((hh, i5, ci))
    ta = sum(chunk_width(i5, ci) for _, i5, ci in A) or 1
    td = sum(chunk_width(i5, ci) for _, i5, ci in Dl) or 1
    out, ia, idx, ca, cd = [], 0, 0, 0.0, 0.0
    while ia < len(A) or idx < len(Dl):
        p = cd / td
        bias = 2048.0 * (1.0 - p) - 1024.0 * p
        fa = (ca / ta) if ia < len(A) else 2.0
        fd = ((cd + bias) / td) if idx < len(Dl) else 2.0
        if fa <= fd:
            out.append(A[ia])
            ca += chunk_width(A[ia][1], A[ia][2])
            ia += 1
        else:
            out.append(Dl[idx])
            cd += chunk_width(Dl[idx][1], Dl[idx][2])
            idx += 1
    return out


def build_nc():
    nc = bacc.Bacc()
    qk = nc.dram_tensor("qk", [HPC, D, 2 * S], mybir.dt.float16, kind="ExternalInput")
    # v layout [head, partition, j, e]: matches the SBUF tile exactly, so the
    # per-head v DMA is one clean 2D transfer (128 x 4128B) instead of 2048
    # tiny strided descriptors eating ~4us of sync-sequencer time per head
    v = nc.dram_tensor("v", [HPC, 128, N_SK, VW], mybir.dt.float16, kind="ExternalInput")
    out = nc.dram_tensor("out", [S, HPC * D], mybir.dt.float32, kind="ExternalOutput")

    with tile.TileContext(nc) as tc:
        with (
            tc.tile_pool(name="const", bufs=1) as constp,
            tc.tile_pool(name="qkp", bufs=4) as qkp,
            tc.tile_pool(name="vp", bufs=3) as vpool,
            tc.tile_pool(name="e", bufs=12) as ep,
            tc.tile_pool(name="stage", bufs=4) as stagep,
            tc.tile_pool(name="rec", bufs=8) as recp,
            tc.tile_pool(name="em", bufs=20) as emp,
            tc.tile_pool(name="ps_s", bufs=RING, space="PSUM") as ps_s,
            tc.tile_pool(name="ps_c", bufs=1, space="PSUM") as ps_c,
        ):
            # tiny dummy exp: triggers the one-time ~2.7us ACT table load
            # during the DMA prologue instead of before the first real exp
            warm = constp.tile([1, 8], mybir.dt.float32, name="warm")
            nc.vector.memset(warm[:], 0.0)
            nc.scalar.activation(
                warm[:], warm[:], mybir.ActivationFunctionType.Exp, scale=EXPSCALE
            )
            # tri[r, c] = 1.0 if c >= r else 0.0 (fp16) - diag mask fallback
            tri = constp.tile([128, 128], mybir.dt.float16)
            nc.gpsimd.memset(tri[:], 1.0)
            nc.gpsimd.affine_select(
                out=tri[:],
                in_=tri[:],
                compare_op=mybir.AluOpType.is_ge,
                fill=0.0,
                base=0,
                pattern=[[1, 128]],
                channel_multiplier=-1,
            )
            if PE_WARMUP:
                # PE warmup: ~3.5us of back-to-back dummy matmuls during the
                # DMA prologue releases the HAM clock gate (PE defaults to
                # half clock until it sees ~3.4us of sustained activity)
                warm_sb = constp.tile([128, 512], mybir.dt.float16, name="warm_sb")
                nc.vector.memset(warm_sb[:], 0.0)
                warm_ps = ps_c.tile(
                    [128, 2 * VW], mybir.dt.float32, tag="cD0", name="warm_ps"
                )
                for _ in range(18):
                    nc.tensor.matmul(
                        warm_ps[:, 0 : 2 * VW],
                        warm_sb[:, 0:128],
                        warm_sb[:, 0 : 2 * VW],
                        start=True,
                        stop=True,
                        skip_group_check=True,
                    )

            started_heads = set()
            vdummy_done = set()
            pending = []  # deferred epilogue units (closures), popped 2/chunk
            qk_holder = {}
            v_holder = {}
            staged_holder = {}
            ctx_holder = {}
            em_holder = {}

            def start_head(hh):
                qk_sb = qkp.tile([128, 2 * S], mybir.dt.float16, tag="qk", name="qk_sb")
                qk_holder[hh] = qk_sb
                v_sb = vpool.tile([128, N_SK * VW], mybir.dt.float16, tag="v", name="v_sb")
                v_holder[hh] = v_sb
                v3 = v_sb[:]
                v3s = v[hh, :, :, :].rearrange("p j e -> p (j e)")
                # qk rides the sync queue; v rides the (mostly idle) gpsimd
                # queue so the two input streams issue + transfer in parallel
                # (the sync sequencer serializing all input DMAs was the
                # cold-start bottleneck that let HAM re-throttle the PE clock)
                if hh == 0 and HEAD0_FEW_DMAS:
                    nc.sync.dma_start(out=qk_sb[:, 0:1024], in_=qk[hh, :, 0:1024])
                    nc.sync.dma_start(out=qk_sb[:, 1024:4096], in_=qk[hh, :, 1024:4096])
                    nc.sync.dma_start(out=v3, in_=v3s)
                else:
                    # later heads are fully prefetched during the previous head
                    nc.sync.dma_start(out=qk_sb[:], in_=qk[hh, :, :])
                    nc.sync.dma_start(out=v3, in_=v3s)
                # staged is VW-strided: each 129-col slot holds a normalized
                # 128-col subtile + its (normalized-to-1, unused) rowsum col,
                # so one batched tensor_mul covers a whole ctx tile
                staged_holder[hh] = stagep.tile(
                    [128, N_SK * VW], mybir.dt.float32, tag="o", name="staged"
                )

            def emit_qk(hh, i5, ci, s_ps, base):
                if hh not in started_heads:
                    start_head(hh)
                    started_heads.add(hh)
                if hh + 1 < HPC and hh + 1 not in started_heads:
                    start_head(hh + 1)
                    started_heads.add(hh + 1)
                qk_sb = qk_holder[hh]
                for j, dst, mv0, w in _CHUNKS[i5][ci]:
                    nc.tensor.matmul(
                        s_ps[:, base + dst : base + dst + w],
                        qk_sb[:, k_col(j) : k_col(j) + 128],
                        qk_sb[:, q_col(i5, mv0) : q_col(i5, mv0) + w],
                        start=True,
                        stop=True,
                    )

            def emit_exp(group, s_ps):
                """One exp instruction covering this group's chunks."""
                aw = (len(group) - 1) * CW + chunk_width(group[-1][1], group[-1][2])
                e_sb = ep.tile([128, len(group) * CW], mybir.dt.float16, tag="e", name="e_sb")
                if is_dve(group[0][0], group[0][1]):
                    # Schraudolph bit-trick exp on DVE: fp16 bits = z + bias,
                    # rounded to int16 on write, bitcast as fp16
                    nc.vector.tensor_scalar(
                        out=e_sb[:, 0:aw].bitcast(mybir.dt.int16),
                        in0=s_ps[:, 0:aw],
                        scalar1=DVE_BIAS,
                        scalar2=None,
                        op0=mybir.AluOpType.add,
                    )
                else:
                    nc.scalar.activation(
                        e_sb[:, 0:aw],
                        s_ps[:, 0:aw],
                        mybir.ActivationFunctionType.Exp,
                        scale=EXPSCALE,
                    )
                # diagonal-subtile masks: keep e where col >= row
                for gi, (hh, i5, ci) in enumerate(group):
                    off = gi * CW
                    for j, dst, mv0, w in _CHUNKS[i5][ci]:
                        for k in range(w // 128):
                            tt = mv0 // 128 + k
                            if tt == j - 4 * i5:
                                em = emp.tile(
                                    [128, 128], mybir.dt.float16, tag="em", name="em"
                                )
                                src = e_sb[:, off + dst + k * 128 : off + dst + (k + 1) * 128]
                                if MASKS_ON_POOL:
                                    nc.gpsimd.affine_select(
                                        out=em[:],
                                        in_=src,
                                        compare_op=mybir.AluOpType.is_ge,
                                        fill=0.0,
                                        base=0,
                                        pattern=[[1, 128]],
                                        channel_multiplier=-1,
                                    )
                                else:
                                    nc.vector.tensor_mul(em[:], src, tri[:])
                                em_holder[(hh, i5, tt)] = em
                return e_sb

            def emit_epilogue_units(hh, i5, ctx):
                staged = staged_holder[hh]

                def recips(ctx=ctx):
                    recs = []
                    for b in range(2):
                        rec2 = recp.tile([128, 2, 1], mybir.dt.float32, tag="rec", name="rec2")
                        c3 = ctx[b].rearrange("p (s v) -> p s v", v=VW)
                        nc.vector.reciprocal(rec2[:], c3[:, :, 128:129])
                        recs.append(rec2)
                    rec_holder[(hh, i5)] = recs

                units = [recips]
                for b in range(2):
                    def norm(b=b, ctx=ctx, staged=staged, hh=hh, i5=i5):
                        # one batched multiply normalizes both subtiles of a
                        # ctx tile (recip broadcast per 129-col group)
                        c3 = ctx[b].rearrange("p (s v) -> p s v", v=VW)
                        dst = staged.rearrange("p (i e) -> p i e", e=VW)[
                            :, i5 * 4 + 2 * b : i5 * 4 + 2 * b + 2, :
                        ]
                        rec_b = rec_holder[(hh, i5)][b].broadcast_to([128, 2, VW])
                        nc.vector.tensor_mul(dst, c3[:], rec_b)
                    units.append(norm)

                def outdma(hh=hh, i5=i5, staged=staged):
                    # alternate DMA queues so neither sequencer backs up; the
                    # last head rides sync only, so the gpsimd SWDGE queue is
                    # empty before the end-of-program drain barrier
                    if hh + 1 == HPC:
                        eng = nc.sync
                    else:
                        eng = nc.gpsimd if (hh * N_I + i5) % 2 == 0 else nc.sync
                    eng.dma_start(
                        out=out[
                            i5 * SQ_BLK : (i5 + 1) * SQ_BLK, hh * D : (hh + 1) * D
                        ].rearrange("(i p) d -> p i d", p=128),
                        in_=staged.rearrange("p (i e) -> p i e", e=VW)[
                            :, i5 * 4 : (i5 + 1) * 4, 0:D
                        ],
                    )

                units.append(outdma)
                pending.extend(units)

            rec_holder = {}

            def emit_pv(hh, i5, ci, e_sb, eoff, is_final_block, is_last_of_block):
                v_sb = v_holder[hh]
                pieces, first, last = _PIECES[i5]
                if ci == 0:
                    # the previous block's epilogue must be fully emitted
                    # before its ctx slots (same tags) are recycled
                    while pending:
                        pending.pop(0)()
                    tags = ("cD0", "cD1") if is_dve(hh, i5) else ("cA0", "cA1")
                    ctx_holder[(hh, i5)] = [
                        ps_c.tile([128, 2 * VW], mybir.dt.float32, tag=tags[b], name=f"ctx{b}")
                        for b in range(2)
                    ]
                ctx = ctx_holder[(hh, i5)]
                if hh not in vdummy_done:
                    # absorb the v-DMA wait on PE right before the head's
                    # first PV matmul (scribbles on ctx; start=True resets)
                    vdummy_done.add(hh)
                    nc.tensor.matmul(
                        ctx[0][0:1, 0:8],
                        v_sb[:, 0:1],
                        v_sb[:, 0:8],
                        start=True,
                        stop=True,
                        skip_group_check=True,
                    )
                for idx, (pci, col, tt, j, dg) in enumerate(pieces):
                    if pci != ci:
                        continue
                    lhs = (
                        em_holder[(hh, i5, tt)][:]
                        if dg
                        else e_sb[:, eoff + col : eoff + col + 128]
                    )
                    nc.tensor.matmul(
                        ctx[tt // 2][:, (tt % 2) * VW : (tt % 2 + 1) * VW],
                        lhs,
                        v_sb[:, j * VW : (j + 1) * VW],
                        start=(idx == first[tt // 2]),
                        stop=(idx == last[tt]),
                        skip_group_check=True,
                    )
                    if is_final_block and tt % 2 == 1 and idx == last[tt]:
                        # final drain, per ctx bank: as soon as both subtiles
                        # of bank tt//2 have accumulated, one batched
                        # normalize + one out-DMA for the 256-row pair
                        staged = staged_holder[hh]
                        b = tt // 2
                        c3 = ctx[b].rearrange("p (s v) -> p s v", v=VW)
                        rec2 = recp.tile(
                            [128, 2, 1], mybir.dt.float32, tag="recf", name="rec2"
                        )
                        nc.vector.reciprocal(rec2[:], c3[:, :, 128:129])
                        dst = staged.rearrange("p (i e) -> p i e", e=VW)[
                            :, i5 * 4 + 2 * b : i5 * 4 + 2 * b + 2, :
                        ]
                        nc.vector.tensor_mul(dst, c3[:], rec2.broadcast_to([128, 2, VW]))
                        nc.sync.dma_start(
                            out=out[
                                (i5 * 4 + 2 * b) * 128 : (i5 * 4 + 2 * b + 2) * 128,
                                hh * D : (hh + 1) * D,
                            ].rearrange("(i p) d -> p i d", p=128),
                            in_=staged.rearrange("p (i e) -> p i e", e=VW)[
                                :, i5 * 4 + 2 * b : i5 * 4 + 2 * b + 2, 0:D
                            ],
                        )
                if is_last_of_block and not is_final_block:
                    emit_epilogue_units(hh, i5, ctx)

            # global chunk order (one chunk per group)
            groups = [[c] for c in build_global_chunks()]
            final_block = (groups[-1][-1][0], groups[-1][-1][1])

            # software-pipelined emission: QK(group), exp(group), PV deferred
            # ~PV_DEFER chunks; between chunks, pop deferred epilogue units
            fifo = []

            def pop_pv():
                (hh, i5, ci), pe_sb, eoff = fifo.pop(0)
                for _ in range(2):
                    if pending:
                        pending.pop(0)()
                emit_pv(
                    hh,
                    i5,
                    ci,
                    pe_sb,
                    eoff,
                    (hh, i5) == final_block,
                    ci == len(_CHUNKS[i5]) - 1,
                )

            for g in groups:
                # PV first: if QK must wait for a ring slot, PE meanwhile
                # has useful PV work in its queue instead of head-of-line
                # blocking on the gated QK
                while len(fifo) >= PV_DEFER:
                    pop_pv()
                s_ps = ps_s.tile([128, CW], mybir.dt.float32, tag="s", name="s_ps")
                for gi, c in enumerate(g):
                    emit_qk(*c, s_ps, gi * CW)
                e_sb = emit_exp(g, s_ps)
                for gi, c in enumerate(g):
                    fifo.append((c, e_sb, gi * CW))
            while fifo:
                pop_pv()
            while pending:
                pending.pop(0)()
    nc.compile()
    return nc


_NC_CACHE = None


def _get_nc():
    global _NC_CACHE
    if _NC_CACHE is None:
        _NC_CACHE = build_nc()
    return _NC_CACHE


def _make_in_maps(query_layer, key_layer, value_layer):
    q = np.asarray(query_layer)
    k = np.asarray(key_layer)
    v = np.asarray(value_layer)
    in_maps = []
    for c in range(N_CORES):
        b = c // 2
        h0 = (c % 2) * HPC
        # [s, h, d] -> [h, d, s], interleaved as four [Q_blk | K_blk]
        # 1024-col slabs; Q pre-scaled by PRE (see module docstring)
        qkc4 = np.empty((HPC, D, N_I, 2, SQ_BLK), dtype=np.float16)
        qkc4[:, :, :, 0, :] = (
            (q[:, b, h0 : h0 + HPC, :] * np.float32(PRE))
            .transpose(1, 2, 0)
            .reshape(HPC, D, N_I, SQ_BLK)
        )
        qkc4[:, :, :, 1, :] = (
            k[:, b, h0 : h0 + HPC, :].transpose(1, 2, 0).reshape(HPC, D, N_I, SQ_BLK)
        )
        qkc = qkc4.reshape(HPC, D, 2 * S)
        # [s, h, d] -> [h, p, j, d] + ones column -> fp16 (v[h, p, j, :d] =
        # V[128j + p, :], matching the SBUF tile layout byte-for-byte)
        vc = np.ones((HPC, 128, N_SK, VW), dtype=np.float16)
        vc[:, :, :, :D] = (
            v[:, b, h0 : h0 + HPC, :]
            .transpose(1, 0, 2)
            .reshape(HPC, N_SK, 128, D)
            .swapaxes(1, 2)
            .astype(np.float16)
        )
        in_maps.append({"qk": qkc, "v": vc})
    return in_maps


def run_spmd(in_maps, **kwargs):
    nc = _get_nc()
    return run_bass_kernel_spmd(nc, in_maps, core_ids=list(range(N_CORES)), **kwargs)


def kernel(query_layer, key_layer, value_layer):
    in_maps = _make_in_maps(query_layer, key_layer, value_layer)
    res = run_spmd(in_maps)
    full = np.empty((S, B, H * D), dtype=np.float32)
    for c in range(N_CORES):
        b = c // 2
        h0 = (c % 2) * HPC
        full[:, b, h0 * D : (h0 + HPC) * D] = res.results[c]["out"]
    return full


# revision 74
# speedup vs baseline: 1.0048x; 1.0048x over previous
"""Causal dot-product attention (s=2048, b=4, h=16, d=128) on 8 TRN2 NeuronCores.

Sharding: batch*heads (64 pairs) split across 8 cores -> 8 (b,h) pairs per core.
Core c handles b = c // 2, heads h in [(c%2)*8, (c%2)*8 + 8).

Dual-engine exp (the baseline was ACT(exp)-throughput-bound at ~146us busy;
this design splits softmax work across ACT, DVE, Pool and PE):
  - ACT blocks (i5 0..2): exp via ACTIVATE (table exp).
  - DVE block (i5=3): exp via the Schraudolph bit-trick: Q is pre-scaled on
    the host by 1024*log2(e)/sqrt(d) so scores z = 1024*log2(e)*s_scaled;
    one DVE tensor_scalar computes int16(round(z + 17408)), bitcast fp16 =
    exp(s_scaled)*4 with a ~1.8% rms periodic ripple.  The 4x cancels in the
    softmax division.  Entire sq-blocks (softmax ROWS) stay on one engine so
    the ripple's constant bias cancels row-wise; only the longest rows
    (i5=3) go to DVE (3.8e-3 rel err measured).
  - A- and D-chunks interleave in program order so both engines stream
    concurrently (the PSUM score ring serializes chunk order, not engines).

Chunking: 512-col score chunks (1 PSUM bank each), ring of 4 + two live ctx
sets (A-block + D-block, 2 banks each) = 8 banks.  Per i5 the column stream
is [diag tiles trimmed to 1024][full j-tiles][the 256-col diag leftover].
PV accumulation flags follow the PSUM group semantics: an accumulation group
commits at stop=True and a start=True discards any open group in the bank,
so start fires only on each bank's first write.

PV is emitted PV_DEFER chunks behind QK/exp, and BEFORE each QK in program
order, so PE always has ready PV work queued while a QK waits for its ring
slot (no head-of-line stall).  Diagonal masks run on the otherwise-idle Pool
engine (affine_select, fp16 SBUF).  Epilogue per block: batched reciprocals
(2 rowsums per ctx tile, strided AP) + ONE broadcast tensor_mul per ctx tile
normalizing both subtiles into a VW-strided staging tile; out-DMAs alternate
between the gpsimd and sync queues so neither sequencer backs up.

Cold start: ~15 dummy PE matmuls release the HAM clock gate (PE boots at
half clock until ~3.4us of sustained activity) during the DMA prologue;
head 0's input is 3 large DMAs; V's DRAM layout matches the SBUF tile
byte-for-byte so each head's V is one clean 2D transfer (the old strided
layout cost ~4us of sync-sequencer descriptor generation per head).

Host-side layout prep: Q scaled by 1024*log2(e)/sqrt(d) and transposed to
[head, d, s] interleaved with K as four [Q_blk | K_blk] 1024-col slabs, fp16.
V cast to fp16 as [head, partition, j, e] with the softmax-denominator ones
column baked in.
"""

import sys

if "/opt/trn_rl_repo" not in sys.path:
    sys.path.insert(0, "/opt/trn_rl_repo")

import numpy as np

import concourse.bacc as bacc
import concourse.bass as bass
import concourse.mybir as mybir
import concourse.tile as tile
from concourse.bass_utils import run_bass_kernel_spmd

S, B, H, D = 2048, 4, 16, 128
N_CORES = 8
HPC = (B * H) // N_CORES  # heads per core = 8

PRE = float(1024.0 * np.log2(np.e) / np.sqrt(128.0))  # folded into Q host-side
EXPSCALE = float(np.log(2.0) / 1024.0)  # ACT exp scale on raw z
# fp16 exponent bias (15*1024) + 2*1024: E = exp(s)*4.  Chosen so the max
# observed score (z=13175, 8.9 sigma) stays below the fp16 inf bit pattern
# (bits < 31744) and the int16 write never saturates; the 4x factor cancels
# in the softmax division.
DVE_BIAS = 17408.0

SQ_BLK = 512
N_I = S // SQ_BLK  # 4 sq blocks per head
N_SK = S // 128  # 16 sk tiles per head
VW = 129  # V tile width incl. ones column
CW = 512  # score chunk width (cols); fp32 -> 2KB -> 1 PSUM bank
RING = 4  # score ring depth (chunks in flight; 4 banks)
PV_DEFER = 8  # PV emitted this many chunks behind QK/exp

DVE_I5 = {3}  # sq blocks whose exp runs on DVE (longest rows: best accuracy)


def is_dve(hh, i5):
    return i5 in DVE_I5
PE_WARMUP = True  # dummy matmuls at t=0 to release the HAM clock gate
HEAD0_FEW_DMAS = True  # head 0 input in 3 large DMAs
MASKS_ON_POOL = True  # diag masks via gpsimd affine_select (else DVE tri mul)


def q_col(i5, c):
    # qk host/SBUF layout: four 1024-col blocks [Q_i5(512) | K_{4*i5..4*i5+3}]
    return i5 * 1024 + c


def k_col(j):
    return (j // 4) * 1024 + 512 + (j % 4) * 128


def pack_chunks(i5):
    """Chunks of (j, dst, mv0, w): QK matmul for sk-tile j writes score cols
    [dst, dst+w) of the chunk from moving Q cols [mv0, mv0+w).  Stream order:
    1024 cols of trimmed diag tiles, then full j-tiles, then the leftover
    256-col diag piece; entries split at chunk (1024) and PSUM-bank (512)
    boundaries."""
    stream = [(4 * i5 + 0, 0, 512), (4 * i5 + 1, 128, 384), (4 * i5 + 3, 384, 128)]
    stream += [(j, 0, 512) for j in range(4 * i5)]
    stream += [(4 * i5 + 2, 256, 256)]
    chunks, cur, used = [], [], 0
    for j, mv0, w in stream:
        while w:
            if used == CW:
                chunks.append(cur)
                cur, used = [], 0
            take = min(w, 512 - used % 512)
            cur.append((j, used, mv0, take))
            used += take
            mv0 += take
            w -= take
    if cur:
        chunks.append(cur)
    return chunks


_CHUNKS = {i5: pack_chunks(i5) for i5 in range(N_I)}


def pieces_of(i5):
    """Ordered PV pieces (ci, col, tt, j, diag) + first/last piece index per
    subtile tt (for matmul start/stop flags)."""
    ps = []
    for ci, ents in enumerate(_CHUNKS[i5]):
        for j, dst, mv0, w in ents:
            for k in range(w // 128):
                tt = mv0 // 128 + k
                ps.append((ci, dst + k * 128, tt, j, tt == j - 4 * i5))
    # PSUM semantics: an accumulation group commits at stop=True; start=True
    # discards any open (un-stopped) group in the bank.  So: start only on
    # each BANK's first write, stop on each subtile's last write.
    first, last = {}, {}
    for idx, (ci, col, tt, j, dg) in enumerate(ps):
        first.setdefault(tt // 2, idx)
        last[tt] = idx
    return ps, first, last


_PIECES = {i5: pieces_of(i5) for i5 in range(N_I)}


def chunk_width(i5, ci):
    return sum(w for _, _, _, w in _CHUNKS[i5][ci])


def build_global_chunks():
    """Global chunk order: fair-merge the A-chunk stream (ACT blocks, all
    heads) and D-chunk stream (DVE blocks, all heads) by fractional column
    progress, so ring alternation never degenerates at head boundaries
    (only ONE stream tail in the whole kernel).  D is delayed ~2048 cols at
    the start (head 0's late slabs) tapering to a small negative shift at
    the end so the final chunks are the last head's small A block (rolling
    drain)."""
    A, Dl = [], []
    for hh in range(HPC):
        order = [0, 1, 2, 3] if hh + 1 < HPC else [3, 2, 1, 0]
        for i5 in order:
            if not is_dve(hh, i5):
                for ci in range(len(_CHUNKS[i5])):
                    A.append((hh, i5, ci))
        for i5 in sorted(i for i in range(N_I) if is_dve(hh, i)):
            for ci in range(len(_CHUNKS[i5])):
                Dl.append((hh, i5, ci))
    ta = sum(chunk_width(i5, ci) for _, i5, ci in A) or 1
    td = sum(chunk_width(i5, ci) for _, i5, ci in Dl) or 1
    out, ia, idx, ca, cd = [], 0, 0, 0.0, 0.0
    while ia < len(A) or idx < len(Dl):
        p = cd / td
        bias = 2048.0 * (1.0 - p) - 1024.0 * p
        fa = (ca / ta) if ia < len(A) else 2.0
        fd = ((cd + bias) / td) if idx < len(Dl) else 2.0
        if fa <= fd:
            out.append(A[ia])
            ca += chunk_width(A[ia][1], A[ia][2])
            ia += 1
        else:
            out.append(Dl[idx])
            cd += chunk_width(Dl[idx][1], Dl[idx][2])
            idx += 1
    return out


def build_nc():
    nc = bacc.Bacc()
    qk = nc.dram_tensor("qk", [HPC, D, 2 * S], mybir.dt.float16, kind="ExternalInput")
    # v layout [head, partition, j, e]: matches the SBUF tile exactly, so the
    # per-head v DMA is one clean 2D transfer (128 x 4128B) instead of 2048
    # tiny strided descriptors eating ~4us of sync-sequencer time per head
    v = nc.dram_tensor("v", [HPC, 128, N_SK, VW], mybir.dt.float16, kind="ExternalInput")
    out = nc.dram_tensor("out", [S, HPC * D], mybir.dt.float32, kind="ExternalOutput")

    with tile.TileContext(nc) as tc:
        with (
            tc.tile_pool(name="const", bufs=1) as constp,
            tc.tile_pool(name="qkp", bufs=4) as qkp,
            tc.tile_pool(name="vp", bufs=3) as vpool,
            tc.tile_pool(name="e", bufs=14) as ep,
            tc.tile_pool(name="stage", bufs=4) as stagep,
            tc.tile_pool(name="rec", bufs=8) as recp,
            tc.tile_pool(name="em", bufs=24) as emp,
            tc.tile_pool(name="ps_s", bufs=RING, space="PSUM") as ps_s,
            tc.tile_pool(name="ps_c", bufs=1, space="PSUM") as ps_c,
        ):
            # tiny dummy exp: triggers the one-time ~2.7us ACT table load
            # during the DMA prologue instead of before the first real exp
            warm = constp.tile([1, 8], mybir.dt.float32, name="warm")
            nc.vector.memset(warm[:], 0.0)
            nc.scalar.activation(
                warm[:], warm[:], mybir.ActivationFunctionType.Exp, scale=EXPSCALE
            )
            # tri[r, c] = 1.0 if c >= r else 0.0 (fp16) - diag mask fallback
            tri = constp.tile([128, 128], mybir.dt.float16)
            nc.gpsimd.memset(tri[:], 1.0)
            nc.gpsimd.affine_select(
                out=tri[:],
                in_=tri[:],
                compare_op=mybir.AluOpType.is_ge,
                fill=0.0,
                base=0,
                pattern=[[1, 128]],
                channel_multiplier=-1,
            )
            if PE_WARMUP:
                # PE warmup: ~3.5us of back-to-back dummy matmuls during the
                # DMA prologue releases the HAM clock gate (PE defaults to
                # half clock until it sees ~3.4us of sustained activity)
                warm_sb = constp.tile([128, 512], mybir.dt.float16, name="warm_sb")
                nc.vector.memset(warm_sb[:], 0.0)
                warm_ps = ps_c.tile(
                    [128, 2 * VW], mybir.dt.float32, tag="cD0", name="warm_ps"
                )
                for _ in range(18):
                    nc.tensor.matmul(
                        warm_ps[:, 0 : 2 * VW],
                        warm_sb[:, 0:128],
                        warm_sb[:, 0 : 2 * VW],
                        start=True,
                        stop=True,
                        skip_group_check=True,
                    )

            started_heads = set()
            vdummy_done = set()
            pending = []  # deferred epilogue units (closures), popped 2/chunk
            qk_holder = {}
            v_holder = {}
            staged_holder = {}
            ctx_holder = {}
            em_holder = {}

            def start_head(hh):
                qk_sb = qkp.tile([128, 2 * S], mybir.dt.float16, tag="qk", name="qk_sb")
                qk_holder[hh] = qk_sb
                v_sb = vpool.tile([128, N_SK * VW], mybir.dt.float16, tag="v", name="v_sb")
                v_holder[hh] = v_sb
                v3 = v_sb[:]
                v3s = v[hh, :, :, :].rearrange("p j e -> p (j e)")
                # qk rides the sync queue; v rides the (mostly idle) gpsimd
                # queue so the two input streams issue + transfer in parallel
                # (the sync sequencer serializing all input DMAs was the
                # cold-start bottleneck that let HAM re-throttle the PE clock)
                if hh == 0 and HEAD0_FEW_DMAS:
                    nc.sync.dma_start(out=qk_sb[:, 0:1024], in_=qk[hh, :, 0:1024])
                    nc.sync.dma_start(out=qk_sb[:, 1024:4096], in_=qk[hh, :, 1024:4096])
                    nc.sync.dma_start(out=v3, in_=v3s)
                else:
                    # later heads are fully prefetched during the previous head
                    nc.sync.dma_start(out=qk_sb[:], in_=qk[hh, :, :])
                    nc.sync.dma_start(out=v3, in_=v3s)
                # staged is VW-strided: each 129-col slot holds a normalized
                # 128-col subtile + its (normalized-to-1, unused) rowsum col,
                # so one batched tensor_mul covers a whole ctx tile
                staged_holder[hh] = stagep.tile(
                    [128, N_SK * VW], mybir.dt.float32, tag="o", name="staged"
                )

            def emit_qk(hh, i5, ci, s_ps, base):
                if hh not in started_heads:
                    start_head(hh)
                    started_heads.add(hh)
                if hh + 1 < HPC and hh + 1 not in started_heads:
                    start_head(hh + 1)
                    started_heads.add(hh + 1)
                qk_sb = qk_holder[hh]
                for j, dst, mv0, w in _CHUNKS[i5][ci]:
                    nc.tensor.matmul(
                        s_ps[:, base + dst : base + dst + w],
                        qk_sb[:, k_col(j) : k_col(j) + 128],
                        qk_sb[:, q_col(i5, mv0) : q_col(i5, mv0) + w],
                        start=True,
                        stop=True,
                    )

            def emit_exp(group, s_ps):
                """One exp instruction covering this group's chunks."""
                aw = (len(group) - 1) * CW + chunk_width(group[-1][1], group[-1][2])
                e_sb = ep.tile([128, len(group) * CW], mybir.dt.float16, tag="e", name="e_sb")
                if is_dve(group[0][0], group[0][1]):
                    # Schraudolph bit-trick exp on DVE: fp16 bits = z + bias,
                    # rounded to int16 on write, bitcast as fp16
                    nc.vector.tensor_scalar(
                        out=e_sb[:, 0:aw].bitcast(mybir.dt.int16),
                        in0=s_ps[:, 0:aw],
                        scalar1=DVE_BIAS,
                        scalar2=None,
                        op0=mybir.AluOpType.add,
                    )
                else:
                    nc.scalar.activation(
                        e_sb[:, 0:aw],
                        s_ps[:, 0:aw],
                        mybir.ActivationFunctionType.Exp,
                        scale=EXPSCALE,
                    )
                # diagonal-subtile masks: keep e where col >= row
                for gi, (hh, i5, ci) in enumerate(group):
                    off = gi * CW
                    for j, dst, mv0, w in _CHUNKS[i5][ci]:
                        for k in range(w // 128):
                            tt = mv0 // 128 + k
                            if tt == j - 4 * i5:
                                em = emp.tile(
                                    [128, 128], mybir.dt.float16, tag="em", name="em"
                                )
                                src = e_sb[:, off + dst + k * 128 : off + dst + (k + 1) * 128]
                                if MASKS_ON_POOL:
                                    nc.gpsimd.affine_select(
                                        out=em[:],
                                        in_=src,
                                        compare_op=mybir.AluOpType.is_ge,
                                        fill=0.0,
                                        base=0,
                                        pattern=[[1, 128]],
                                        channel_multiplier=-1,
                                    )
                                else:
                                    nc.vector.tensor_mul(em[:], src, tri[:])
                                em_holder[(hh, i5, tt)] = em
                return e_sb

            def emit_epilogue_units(hh, i5, ctx):
                staged = staged_holder[hh]

                def recips(ctx=ctx):
                    recs = []
                    for b in range(2):
                        rec2 = recp.tile([128, 2, 1], mybir.dt.float32, tag="rec", name="rec2")
                        c3 = ctx[b].rearrange("p (s v) -> p s v", v=VW)
                        nc.vector.reciprocal(rec2[:], c3[:, :, 128:129])
                        recs.append(rec2)
                    rec_holder[(hh, i5)] = recs

                units = [recips]
                for b in range(2):
                    def norm(b=b, ctx=ctx, staged=staged, hh=hh, i5=i5):
                        # one batched multiply normalizes both subtiles of a
                        # ctx tile (recip broadcast per 129-col group)
                        c3 = ctx[b].rearrange("p (s v) -> p s v", v=VW)
                        dst = staged.rearrange("p (i e) -> p i e", e=VW)[
                            :, i5 * 4 + 2 * b : i5 * 4 + 2 * b + 2, :
                        ]
                        rec_b = rec_holder[(hh, i5)][b].broadcast_to([128, 2, VW])
                        nc.vector.tensor_mul(dst, c3[:], rec_b)
                    units.append(norm)

                def outdma(hh=hh, i5=i5, staged=staged):
                    # alternate DMA queues so neither sequencer backs up; the
                    # last head rides sync only, so the gpsimd SWDGE queue is
                    # empty before the end-of-program drain barrier
                    if hh + 1 == HPC:
                        eng = nc.sync
                    else:
                        eng = nc.gpsimd if (hh * N_I + i5) % 2 == 0 else nc.sync
                    eng.dma_start(
                        out=out[
                            i5 * SQ_BLK : (i5 + 1) * SQ_BLK, hh * D : (hh + 1) * D
                        ].rearrange("(i p) d -> p i d", p=128),
                        in_=staged.rearrange("p (i e) -> p i e", e=VW)[
                            :, i5 * 4 : (i5 + 1) * 4, 0:D
                        ],
                    )

                units.append(outdma)
                pending.extend(units)

            rec_holder = {}

            def emit_pv(hh, i5, ci, e_sb, eoff, is_final_block, is_last_of_block):
                v_sb = v_holder[hh]
                pieces, first, last = _PIECES[i5]
                if ci == 0:
                    # the previous block's epilogue must be fully emitted
                    # before its ctx slots (same tags) are recycled
                    while pending:
                        pending.pop(0)()
                    tags = ("cD0", "cD1") if is_dve(hh, i5) else ("cA0", "cA1")
                    ctx_holder[(hh, i5)] = [
                        ps_c.tile([128, 2 * VW], mybir.dt.float32, tag=tags[b], name=f"ctx{b}")
                        for b in range(2)
                    ]
                ctx = ctx_holder[(hh, i5)]
                if hh not in vdummy_done:
                    # absorb the v-DMA wait on PE right before the head's
                    # first PV matmul (scribbles on ctx; start=True resets)
                    vdummy_done.add(hh)
                    nc.tensor.matmul(
                        ctx[0][0:1, 0:8],
                        v_sb[:, 0:1],
                        v_sb[:, 0:8],
                        start=True,
                        stop=True,
                        skip_group_check=True,
                    )
                for idx, (pci, col, tt, j, dg) in enumerate(pieces):
                    if pci != ci:
                        continue
                    lhs = (
                        em_holder[(hh, i5, tt)][:]
                        if dg
                        else e_sb[:, eoff + col : eoff + col + 128]
                    )
                    nc.tensor.matmul(
                        ctx[tt // 2][:, (tt % 2) * VW : (tt % 2 + 1) * VW],
                        lhs,
                        v_sb[:, j * VW : (j + 1) * VW],
                        start=(idx == first[tt // 2]),
                        stop=(idx == last[tt]),
                        skip_group_check=True,
                    )
                    if is_final_block and tt % 2 == 1 and idx == last[tt]:
                        # final drain, per ctx bank: as soon as both subtiles
                        # of bank tt//2 have accumulated, one batched
                        # normalize + one out-DMA for the 256-row pair
                        staged = staged_holder[hh]
                        b = tt // 2
                        c3 = ctx[b].rearrange("p (s v) -> p s v", v=VW)
                        rec2 = recp.tile(
                            [128, 2, 1], mybir.dt.float32, tag="recf", name="rec2"
                        )
                        nc.vector.reciprocal(rec2[:], c3[:, :, 128:129])
                        dst = staged.rearrange("p (i e) -> p i e", e=VW)[
                            :, i5 * 4 + 2 * b : i5 * 4 + 2 * b + 2, :
                        ]
                        nc.vector.tensor_mul(dst, c3[:], rec2.broadcast_to([128, 2, VW]))
                        nc.sync.dma_start(
                            out=out[
                                (i5 * 4 + 2 * b) * 128 : (i5 * 4 + 2 * b + 2) * 128,
                                hh * D : (hh + 1) * D,
                            ].rearrange("(i p) d -> p i d", p=128),
                            in_=staged.rearrange("p (i e) -> p i e", e=VW)[
                                :, i5 * 4 + 2 * b : i5 * 4 + 2 * b + 2, 0:D
                            ],
                        )
                if is_last_of_block and not is_final_block:
                    emit_epilogue_units(hh, i5, ctx)

            # global chunk order (one chunk per group)
            groups = [[c] for c in build_global_chunks()]
            final_block = (groups[-1][-1][0], groups[-1][-1][1])

            # software-pipelined emission: QK(group), exp(group), PV deferred
            # ~PV_DEFER chunks; between chunks, pop deferred epilogue units
            fifo = []

            def pop_pv():
                (hh, i5, ci), pe_sb, eoff = fifo.pop(0)
                for _ in range(2):
                    if pending:
                        pending.pop(0)()
                emit_pv(
                    hh,
                    i5,
                    ci,
                    pe_sb,
                    eoff,
                    (hh, i5) == final_block,
                    ci == len(_CHUNKS[i5]) - 1,
                )

            for g in groups:
                # PV first: if QK must wait for a ring slot, PE meanwhile
                # has useful PV work in its queue instead of head-of-line
                # blocking on the gated QK
                while len(fifo) >= PV_DEFER:
                    pop_pv()
                s_ps = ps_s.tile([128, CW], mybir.dt.float32, tag="s", name="s_ps")
                for gi, c in enumerate(g):
                    emit_qk(*c, s_ps, gi * CW)
                e_sb = emit_exp(g, s_ps)
                for gi, c in enumerate(g):
                    fifo.append((c, e_sb, gi * CW))
            while fifo:
                pop_pv()
            while pending:
                pending.pop(0)()
    nc.compile()
    return nc


_NC_CACHE = None


def _get_nc():
    global _NC_CACHE
    if _NC_CACHE is None:
        _NC_CACHE = build_nc()
    return _NC_CACHE


def _make_in_maps(query_layer, key_layer, value_layer):
    q = np.asarray(query_layer)
    k = np.asarray(key_layer)
    v = np.asarray(value_layer)
    in_maps = []
    for c in range(N_CORES):
        b = c // 2
        h0 = (c % 2) * HPC
        # [s, h, d] -> [h, d, s], interleaved as four [Q_blk | K_blk]
        # 1024-col slabs; Q pre-scaled by PRE (see module docstring)
        qkc4 = np.empty((HPC, D, N_I, 2, SQ_BLK), dtype=np.float16)
        qkc4[:, :, :, 0, :] = (
            (q[:, b, h0 : h0 + HPC, :] * np.float32(PRE))
            .transpose(1, 2, 0)
            .reshape(HPC, D, N_I, SQ_BLK)
        )
        qkc4[:, :, :, 1, :] = (
            k[:, b, h0 : h0 + HPC, :].transpose(1, 2, 0).reshape(HPC, D, N_I, SQ_BLK)
        )
        qkc = qkc4.reshape(HPC, D, 2 * S)
        # [s, h, d] -> [h, p, j, d] + ones column -> fp16 (v[h, p, j, :d] =
        # V[128j + p, :], matching the SBUF tile layout byte-for-byte)
        vc = np.ones((HPC, 128, N_SK, VW), dtype=np.float16)
        vc[:, :, :, :D] = (
            v[:, b, h0 : h0 + HPC, :]
            .transpose(1, 0, 2)
            .reshape(HPC, N_SK, 128, D)
            .swapaxes(1, 2)
            .astype(np.float16)
        )
        in_maps.append({"qk": qkc, "v": vc})
    return in_maps


def run_spmd(in_maps, **kwargs):
    nc = _get_nc()
    return run_bass_kernel_spmd(nc, in_maps, core_ids=list(range(N_CORES)), **kwargs)


def kernel(query_layer, key_layer, value_layer):
    in_maps = _make_in_maps(query_layer, key_layer, value_layer)
    res = run_spmd(in_maps)
    full = np.empty((S, B, H * D), dtype=np.float32)
    for c in range(N_CORES):
        b = c // 2
        h0 = (c % 2) * HPC
        full[:, b, h0 * D : (h0 + HPC) * D] = res.results[c]["out"]
    return full
